# revision 1
# baseline (speedup 1.0000x reference)
"""3-layer GAT + per-graph mean-pool + linear head, distributed over 8 NeuronCores.

Strategy (edge-parallel, dst-sorted):
  * Host: sort edges by dst; each core owns a contiguous dst range of
    N/8 = 2560 nodes (= 8 whole graphs), split into 20 windows of 128 dst
    nodes.  Window edge lists are padded (src=0, dst_local=300) to a uniform
    number of 128-edge blocks (NBLK, global max) so one SPMD program fits
    all 8 cores; per-core behavior differs only through index inputs.
  * Per layer the device builds a node table  z_ext[n] = [z(256) | el(4) | er(4)]
    f32 (el/er are the attention logits, folded into the layer matmul via
    Wel = W @ albd, Wer = W @ arbd).  Layer 0's table is computed fully
    replicated on every core (h = x is an input); layers 1-2 compute the
    local 2560-row slice and AllGather the full table.
  * Edge phase per window: one [128,1]-indexed indirect-DMA gather per
    128-edge block pulls z_ext[src] rows (the only indirect-DMA form this
    runtime supports); er[dst] is fetched once per window for the 128 owned
    nodes and expanded to edges with a one-hot SelT matmul.  Softmax:
    ex = exp(leaky_relu(el+er)) batched per window (DVE + one ACT op);
    messages are scaled in place and scatter-added via one-hot Sel matmuls
    accumulating [out | sum_exp] in PSUM.  Per-node normalization (out/s)
    happens AFTER aggregation; the segment-max shift is dropped (softmax is
    shift-invariant and these logits cannot overflow exp in f32).
  * Pooling: per-window graph-membership one-hot matmul accumulates graph
    sums; each core emits logits for its own 8 graphs; host concatenates.
"""

import sys

import numpy as np

sys.path.insert(0, "/opt/trn_rl_repo")

import concourse.bass as bass
import concourse.bacc as bacc
import concourse.mybir as mybir
import concourse.tile as tile
from concourse.bass_utils import run_bass_kernel_spmd
from concourse.masks import make_identity

# Problem shape (hardcoded per contest rules).
N, E, G = 20480, 327680, 64
IN_DIM, H, D, C = 128, 4, 64, 10
HD = H * D            # 256
ROW = HD + 2 * H      # 264 = z | el | er
NCORES = 8
RN = N // NCORES      # 2560 dst nodes per core
P = 128
NW = RN // P          # 20 windows per core
G8 = G // NCORES      # 8 graphs per core
NEG_SLOPE = 0.2
F32 = mybir.dt.float32
I32 = mybir.dt.int32

TRACE = False         # set by test.py to capture HW profile
LAST_EXEC_NS = None
LAST_RESULTS = None

_CACHE = {}


def _install_ntff_hook_shim():
    """This image's ``antenv`` lacks ``axon_hooks``; provide the thin ctypes
    shim around libaxon_pjrt.so so run_bass_kernel_spmd(trace=True) works."""
    try:
        import antenv.axon_hooks  # noqa: F401
        return
    except ImportError:
        pass
    import contextlib
    import ctypes
    import types

    so_path = "/opt/axon/libaxon_pjrt.so"
    try:
        lib = ctypes.CDLL(so_path)
    except OSError:
        return
    if not hasattr(lib, "axon_start_nrt_profile"):
        return
    lib.axon_start_nrt_profile.argtypes = [ctypes.POINTER(ctypes.c_int64), ctypes.c_size_t]
    lib.axon_start_nrt_profile.restype = ctypes.c_int64
    lib.axon_stop_nrt_profile.argtypes = [ctypes.c_char_p]
    lib.axon_stop_nrt_profile.restype = ctypes.c_int64

    @contextlib.contextmanager
    def _hook(output_dir, device_ids):
        import jax

        jax.devices()
        if device_ids:
            ids = (ctypes.c_int64 * len(device_ids))(*device_ids)
            rc = lib.axon_start_nrt_profile(ids, len(device_ids))
        else:
            rc = lib.axon_start_nrt_profile(None, 0)
        if rc != 0:
            raise RuntimeError(f"axon_start_nrt_profile rc={rc}")
        try:
            yield
        finally:
            n = lib.axon_stop_nrt_profile(str(output_dir).encode())
            print(f"ntff profile: {n} file(s) written to {output_dir}")

    mod = types.ModuleType("antenv.axon_hooks")
    mod.get_axon_ntff_profile_hook = lambda: _hook
    mod.set_axon_ntff_profile_hook = lambda h: None
    sys.modules["antenv.axon_hooks"] = mod


# ----------------------------------------------------------------------------
# Host-side index preprocessing (layout only -- no arithmetic on tensor data)
# ----------------------------------------------------------------------------
def _host_prep(src, dst, graph_ids):
    order = np.argsort(dst, kind="stable")
    src_s = src[order].astype(np.int64)
    dst_s = dst[order].astype(np.int64)
    win = dst_s // P                              # global window 0..159
    cnt = np.bincount(win, minlength=NCORES * NW)
    nblk = int(np.ceil(cnt.max() / P))
    slots = nblk * P

    starts = np.zeros(NCORES * NW, np.int64)
    starts[1:] = np.cumsum(cnt)[:-1]
    srcidx = np.zeros((NCORES * NW, slots), np.int32)            # pad -> row 0
    dstloc = np.full((NCORES * NW, slots), 300.0, np.float32)    # pad -> no match
    for w in range(NCORES * NW):
        c0, c1 = starts[w], starts[w] + cnt[w]
        srcidx[w, : cnt[w]] = src_s[c0:c1]
        dstloc[w, : cnt[w]] = (dst_s[c0:c1] - w * P).astype(np.float32)

    NB = NW * nblk

    def to_cols(a, dt):
        # [160, slots] -> per-core [128, NW*nblk]; (p, w*nblk+b) = edge b*128+p
        a = a.reshape(NCORES, NW, nblk, P)
        a = np.transpose(a, (0, 3, 1, 2))
        return [
            np.ascontiguousarray(a[c].reshape(P, NB).astype(dt))
            for c in range(NCORES)
        ]

    srcidx_d = to_cols(srcidx, np.int32)
    dstloc_d = to_cols(dstloc, np.float32)
    ownid_d = [
        np.ascontiguousarray(
            (c * RN + np.arange(NW)[None, :] * P + np.arange(P)[:, None]).astype(np.int32)
        )
        for c in range(NCORES)
    ]

    gids = np.asarray(graph_ids).astype(np.int64).reshape(NCORES, NW, P)
    gmask = []
    for c in range(NCORES):
        m = np.zeros((P, NW * G8), np.float32)
        for w in range(NW):
            loc = gids[c, w] - c * G8              # 0..7 within this core
            m[np.arange(P), w * G8 + loc] = 1.0
        gmask.append(m)
    return nblk, srcidx_d, dstloc_d, ownid_d, gmask


def _blockdiag(a):
    # [H, D] -> [HD, H] block-diagonal layout so  el = z @ a_bd
    out = np.zeros((HD, H), np.float32)
    for h in range(H):
        out[h * D : (h + 1) * D, h] = a[h]
    return out


# ----------------------------------------------------------------------------
# Device program
# ----------------------------------------------------------------------------
def _build_program(nblk):
    NB = NW * nblk
    nc = bacc.Bacc(
        "TRN2",
        target_bir_lowering=False,
        debug=False,
        enable_asserts=False,
        num_devices=NCORES,
    )

    xT = nc.dram_tensor("xT", [IN_DIM, N], F32, kind="ExternalInput")
    Ws, WTs, ALs, ARs = [], [], [], []
    for l, K in enumerate([IN_DIM, HD, HD]):
        Ws.append(nc.dram_tensor(f"W{l}", [K, HD], F32, kind="ExternalInput"))
        WTs.append(nc.dram_tensor(f"WT{l}", [HD, K], F32, kind="ExternalInput"))
        ALs.append(nc.dram_tensor(f"albd{l}", [HD, H], F32, kind="ExternalInput"))
        ARs.append(nc.dram_tensor(f"arbd{l}", [HD, H], F32, kind="ExternalInput"))
    Wc = nc.dram_tensor("Wc", [HD, C], F32, kind="ExternalInput")
    bc = nc.dram_tensor("bc_rep", [G8, C], F32, kind="ExternalInput")
    srci = nc.dram_tensor("srcidx", [P, NB], I32, kind="ExternalInput")
    dstl = nc.dram_tensor("dstloc", [P, NB], F32, kind="ExternalInput")
    owni = nc.dram_tensor("ownid", [P, NW], I32, kind="ExternalInput")
    gmk = nc.dram_tensor("gmask", [P, NW * G8], F32, kind="ExternalInput")
    logits = nc.dram_tensor("logits", [G8, C], F32, kind="ExternalOutput")

    ztab = [
        nc.dram_tensor("ztab0", [N, ROW], F32),
        nc.dram_tensor("ztab1", [N, ROW], F32),
        nc.dram_tensor("ztab2", [N, ROW], F32),
    ]
    zsl = [
        None,
        nc.dram_tensor("zsl1", [RN, ROW], F32),
        nc.dram_tensor("zsl2", [RN, ROW], F32),
    ]

    AL = mybir.AluOpType

    with tile.TileContext(nc) as tc:
        with (
            tc.tile_pool(name="const", bufs=1) as constp,
            tc.tile_pool(name="wext", bufs=2) as wextp,
            tc.tile_pool(name="mm", bufs=3) as mmp,
            tc.tile_pool(name="edge", bufs=2) as edgep,
            tc.tile_pool(name="sel", bufs=2 * nblk + 2) as selp,
            tc.tile_pool(name="small", bufs=4) as smallp,
            tc.tile_pool(name="psmm", bufs=3, space="PSUM") as psmm,
            tc.tile_pool(name="psel", bufs=2, space="PSUM") as pselp,
            tc.tile_pool(name="psedge", bufs=2, space="PSUM") as psedge,
            tc.tile_pool(name="pshg", bufs=1, space="PSUM") as pshg,
        ):
            # ---- constants / resident state ----
            ident = constp.tile([P, P], F32, tag="ident")
            make_identity(nc, ident[:])
            iota_i = constp.tile([P, P], I32, tag="iota_i")
            nc.gpsimd.iota(iota_i[:], pattern=[[1, P]], base=0, channel_multiplier=0)
            iota_f = constp.tile([P, P], F32, tag="iota_f")
            nc.vector.tensor_copy(iota_f[:], iota_i[:])
            srci_sb = constp.tile([P, NB], I32, tag="srci")
            nc.sync.dma_start(srci_sb[:], srci[:, :])
            dstl_sb = constp.tile([P, NB], F32, tag="dstl")
            nc.sync.dma_start(dstl_sb[:], dstl[:, :])
            owni_sb = constp.tile([P, NW], I32, tag="owni")
            nc.sync.dma_start(owni_sb[:], owni[:, :])
            gmk_sb = constp.tile([P, NW * G8], F32, tag="gmk")
            nc.sync.dma_start(gmk_sb[:], gmk[:, :])
            h_all = constp.tile([P, NW, HD], F32, tag="h_all")
            hg_acc = constp.tile([G8, HD], F32, tag="hg_acc")
            nc.gpsimd.memset(hg_acc[:], 0.0)

            def elu_into(dst_ap, src_ap):
                # elu(x) = max(x,0) + (exp(min(x,0)) - 1)
                mn = mmp.tile([P, HD], F32, tag="emn")
                nc.vector.tensor_scalar_min(mn[:], src_ap, 0.0)
                ex = mmp.tile([P, HD], F32, tag="eex")
                nc.scalar.activation(ex[:], mn[:], mybir.ActivationFunctionType.Exp)
                mx = mmp.tile([P, HD], F32, tag="emx")
                nc.vector.tensor_scalar_max(mx[:], src_ap, 0.0)
                nc.vector.tensor_scalar_add(ex[:], ex[:], -1.0)
                nc.vector.tensor_tensor(out=dst_ap, in0=ex[:], in1=mx[:], op=AL.add)

            def build_wext(l, K):
                kch = K // P
                och = HD // P
                W_sb, WT_sb, al_sb, ar_sb = [], [], [], []
                for k in range(kch):
                    t = wextp.tile([P, HD], F32, tag="wld")
                    nc.sync.dma_start(t[:], Ws[l][k * P : (k + 1) * P, :])
                    W_sb.append(t)
                for oc in range(och):
                    t = wextp.tile([P, K], F32, tag="wtld")
                    nc.sync.dma_start(t[:], WTs[l][oc * P : (oc + 1) * P, :])
                    WT_sb.append(t)
                    ta = wextp.tile([P, H], F32, tag="alld")
                    nc.sync.dma_start(ta[:], ALs[l][oc * P : (oc + 1) * P, :])
                    al_sb.append(ta)
                    tr = wextp.tile([P, H], F32, tag="arld")
                    nc.sync.dma_start(tr[:], ARs[l][oc * P : (oc + 1) * P, :])
                    ar_sb.append(tr)
                wext = []
                for k in range(kch):
                    wx = wextp.tile([P, ROW], F32, tag="wext")
                    nc.vector.tensor_copy(wx[:, 0:HD], W_sb[k][:])
                    for dstcol, bd in ((HD, al_sb), (HD + H, ar_sb)):
                        ps = psmm.tile([P, H], F32, tag="mm")
                        for oc in range(och):
                            nc.tensor.matmul(
                                ps[:],
                                lhsT=WT_sb[oc][:, k * P : (k + 1) * P],
                                rhs=bd[oc][:],
                                start=(oc == 0),
                                stop=(oc == och - 1),
                            )
                        nc.vector.tensor_copy(wx[:, dstcol : dstcol + H], ps[:])
                    wext.append(wx)
                return wext

            def l0_table(wext):
                for t in range(N // P):
                    xt = mmp.tile([P, P], F32, tag="xt")
                    nc.sync.dma_start(xt[:], xT[:, t * P : (t + 1) * P])
                    zp = psmm.tile([P, ROW], F32, tag="mm")
                    nc.tensor.matmul(zp[:], lhsT=xt[:], rhs=wext[0][:], start=True, stop=True)
                    zs = mmp.tile([P, ROW], F32, tag="zs")
                    nc.vector.tensor_copy(zs[:], zp[:])
                    nc.sync.dma_start(ztab[0][t * P : (t + 1) * P, :], zs[:])

            def lx_slice(l, wext):
                for w in range(NW):
                    hts = []
                    for c2 in range(2):
                        tp = psmm.tile([P, P], F32, tag="mm")
                        nc.tensor.transpose(
                            tp[:], h_all[:, w, c2 * P : (c2 + 1) * P], ident[:]
                        )
                        ht = mmp.tile([P, P], F32, tag="ht")
                        nc.vector.tensor_copy(ht[:], tp[:])
                        hts.append(ht)
                    zp = psmm.tile([P, ROW], F32, tag="mm")
                    for c2 in range(2):
                        nc.tensor.matmul(
                            zp[:],
                            lhsT=hts[c2][:],
                            rhs=wext[c2][:],
                            start=(c2 == 0),
                            stop=(c2 == 1),
                        )
                    zs = mmp.tile([P, ROW], F32, tag="zs")
                    nc.vector.tensor_copy(zs[:], zp[:])
                    nc.sync.dma_start(zsl[l][w * P : (w + 1) * P, :], zs[:])
                nc.gpsimd.collective_compute(
                    "AllGather",
                    AL.bypass,
                    replica_groups=[list(range(NCORES))],
                    ins=[zsl[l][:, :]],
                    outs=[ztab[l][:, :]],
                )

            def edge_phase(l):
                for w in range(NW):
                    # er for the 128 owned dst nodes of this window
                    erw = smallp.tile([P, H], F32, tag="erw")
                    nc.gpsimd.indirect_dma_start(
                        out=erw[:],
                        out_offset=None,
                        in_=ztab[l][:, :],
                        in_offset=bass.IndirectOffsetOnAxis(
                            ap=owni_sb[:, w : w + 1], axis=0
                        ),
                        element_offset=HD + H,
                    )
                    zel = edgep.tile([P, nblk, ROW], F32, tag="zel")
                    eall = smallp.tile([P, nblk, H], F32, tag="eall")
                    sels = []
                    for b in range(nblk):
                        g = w * nblk + b
                        nc.gpsimd.indirect_dma_start(
                            out=zel[:, b, :],
                            out_offset=None,
                            in_=ztab[l][:, :],
                            in_offset=bass.IndirectOffsetOnAxis(
                                ap=srci_sb[:, g : g + 1], axis=0
                            ),
                        )
                        # one-hot Sel (edges x dst-nodes); also used for the scatter
                        sel = selp.tile([P, P], F32, tag="sel")
                        nc.vector.tensor_scalar(
                            out=sel[:], in0=iota_f[:],
                            scalar1=dstl_sb[:, g : g + 1], scalar2=None,
                            op0=AL.is_equal,
                        )
                        sels.append(sel)
                        # er[dst] expansion: SelT = transpose(Sel); er_edges = SelT.T @ erw
                        stp = pselp.tile([P, P], F32, tag="psel")
                        nc.tensor.transpose(stp[:], sel[:], ident[:])
                        selt = selp.tile([P, P], F32, tag="selt")
                        nc.vector.tensor_copy(selt[:], stp[:])
                        erps = pselp.tile([P, H], F32, tag="psel")
                        nc.tensor.matmul(
                            erps[:], lhsT=selt[:], rhs=erw[:], start=True, stop=True
                        )
                        nc.vector.tensor_tensor(
                            out=eall[:, b, :], in0=zel[:, b, HD : HD + H],
                            in1=erps[:], op=AL.add,
                        )
                    # batched leaky-relu + exp for the whole window
                    et = smallp.tile([P, nblk, H], F32, tag="et")
                    nc.vector.tensor_scalar_mul(et[:], eall[:], NEG_SLOPE)
                    nc.vector.tensor_tensor(out=eall[:], in0=eall[:], in1=et[:], op=AL.max)
                    nc.scalar.activation(
                        zel[:, :, HD : HD + H], eall[:],
                        mybir.ActivationFunctionType.Exp,
                    )
                    outp = psedge.tile([P, HD + H], F32, tag="outp")
                    for b in range(nblk):
                        sel = sels[b]
                        zb = zel[:, b, 0:HD].rearrange("p (h d) -> p h d", h=H)
                        nc.vector.tensor_tensor(
                            out=zb, in0=zb,
                            in1=zel[:, b, HD : HD + H].to_broadcast([P, H, D]),
                            op=AL.mult,
                        )
                        nc.tensor.matmul(
                            outp[:],
                            lhsT=sel[:],
                            rhs=zel[:, b, 0 : HD + H],
                            start=(b == 0),
                            stop=(b == nblk - 1),
                        )
                    # normalize + activations
                    rec = smallp.tile([P, H], F32, tag="rec")
                    nc.vector.reciprocal(rec[:], outp[:, HD : HD + H])
                    agg = mmp.tile([P, HD], F32, tag="agg")
                    nc.vector.tensor_tensor(
                        out=agg[:].rearrange("p (h d) -> p h d", h=H),
                        in0=outp[:, 0:HD].rearrange("p (h d) -> p h d", h=H),
                        in1=rec[:].to_broadcast([P, H, D]),
                        op=AL.mult,
                    )
                    if l == 0:
                        elu_into(h_all[:, w, :], agg[:])
                    else:
                        nc.vector.tensor_tensor(
                            out=agg[:], in0=agg[:], in1=h_all[:, w, :], op=AL.add
                        )
                        tmp = mmp.tile([P, HD], F32, tag="agg2")
                        elu_into(tmp[:], agg[:])
                        elu_into(h_all[:, w, :], tmp[:])
                    if l == 2:
                        gp = pshg.tile([G8, HD], F32, tag="hg")
                        nc.tensor.matmul(
                            gp[:],
                            lhsT=gmk_sb[:, w * G8 : (w + 1) * G8],
                            rhs=h_all[:, w, :],
                            start=True,
                            stop=True,
                        )
                        nc.vector.tensor_tensor(
                            out=hg_acc[:], in0=hg_acc[:], in1=gp[:], op=AL.add
                        )

            # ---- layer 0 ----
            wext0 = build_wext(0, IN_DIM)
            l0_table(wext0)
            edge_phase(0)
            # ---- layers 1, 2 ----
            for l in (1, 2):
                wextl = build_wext(l, HD)
                lx_slice(l, wextl)
                edge_phase(l)

            # ---- pooling epilogue: hg -> elu -> @Wc + bc ----
            hg_sb = smallp.tile([G8, HD], F32, tag="hg_sb")
            nc.vector.tensor_scalar_mul(hg_sb[:], hg_acc[:], 1.0 / (N // G))
            mn = smallp.tile([G8, HD], F32, tag="fmn")
            nc.vector.tensor_scalar_min(mn[:], hg_sb[:], 0.0)
            exx = smallp.tile([G8, HD], F32, tag="fex")
            nc.scalar.activation(exx[:], mn[:], mybir.ActivationFunctionType.Exp)
            mx = smallp.tile([G8, HD], F32, tag="fmx")
            nc.vector.tensor_scalar_max(mx[:], hg_sb[:], 0.0)
            nc.vector.tensor_scalar_add(exx[:], exx[:], -1.0)
            nc.vector.tensor_tensor(out=hg_sb[:], in0=exx[:], in1=mx[:], op=AL.add)

            wc_sb, hgts = [], []
            for c2 in range(2):
                t = smallp.tile([P, C], F32, tag="wc")
                nc.sync.dma_start(t[:], Wc[c2 * P : (c2 + 1) * P, :])
                wc_sb.append(t)
                tp = psmm.tile([P, G8], F32, tag="mm")
                nc.tensor.transpose(
                    tp[:], hg_sb[:, c2 * P : (c2 + 1) * P], ident[:G8, :G8]
                )
                hgt = smallp.tile([P, G8], F32, tag="hgt")
                nc.vector.tensor_copy(hgt[:], tp[:])
                hgts.append(hgt)
            lg = psmm.tile([G8, C], F32, tag="mm")
            for c2 in range(2):
                nc.tensor.matmul(
                    lg[:], lhsT=hgts[c2][:], rhs=wc_sb[c2][:],
                    start=(c2 == 0), stop=(c2 == 1),
                )
            bc_sb = smallp.tile([G8, C], F32, tag="bc")
            nc.sync.dma_start(bc_sb[:], bc[:, :])
            lg_sb = smallp.tile([G8, C], F32, tag="lg")
            nc.vector.tensor_tensor(out=lg_sb[:], in0=lg[:], in1=bc_sb[:], op=AL.add)
            nc.sync.dma_start(logits[:, :], lg_sb[:])

    nc.compile()
    return nc


def _get_program(nblk):
    if nblk not in _CACHE:
        _CACHE[nblk] = _build_program(nblk)
    return _CACHE[nblk]


# ----------------------------------------------------------------------------
# Entry point
# ----------------------------------------------------------------------------
def kernel(x, src, dst, graph_ids, W0, al0, ar0, W1, al1, ar1, W2, al2, ar2, Wc, bc):
    global LAST_EXEC_NS, LAST_RESULTS
    x = np.ascontiguousarray(np.asarray(x, np.float32))
    src = np.asarray(src).astype(np.int32)
    dst = np.asarray(dst).astype(np.int32)
    graph_ids = np.asarray(graph_ids).astype(np.int32)

    nblk, srcidx_d, dstloc_d, ownid_d, gmask_d = _host_prep(src, dst, graph_ids)
    nc = _get_program(nblk)

    xT = np.ascontiguousarray(x.T)
    Wl = [np.asarray(W0, np.float32), np.asarray(W1, np.float32), np.asarray(W2, np.float32)]
    als = [al0, al1, al2]
    ars = [ar0, ar1, ar2]
    common = {"xT": xT, "Wc": np.asarray(Wc, np.float32),
              "bc_rep": np.tile(np.asarray(bc, np.float32)[None, :], (G8, 1))}
    for l in range(3):
        common[f"W{l}"] = Wl[l]
        common[f"WT{l}"] = np.ascontiguousarray(Wl[l].T)
        common[f"albd{l}"] = _blockdiag(np.asarray(als[l], np.float32))
        common[f"arbd{l}"] = _blockdiag(np.asarray(ars[l], np.float32))

    in_maps = []
    for c in range(NCORES):
        m = dict(common)
        m["srcidx"] = srcidx_d[c]
        m["dstloc"] = dstloc_d[c]
        m["ownid"] = ownid_d[c]
        m["gmask"] = gmask_d[c]
        in_maps.append(m)

    if TRACE:
        _install_ntff_hook_shim()
    res = run_bass_kernel_spmd(nc, in_maps, list(range(NCORES)), trace=TRACE)
    LAST_EXEC_NS = res.exec_time_ns
    LAST_RESULTS = res
    out = np.concatenate([res.results[c]["logits"] for c in range(NCORES)], axis=0)
    return out.astype(np.float32)



# revision 8
# speedup vs baseline: 1.0281x; 1.0281x over previous
"""3-layer GAT + per-graph mean-pool + linear head, distributed over 8 NeuronCores.

Strategy (edge-parallel, dst-sorted):
  * Host: sort edges by dst; each core owns a contiguous dst range of
    N/8 = 2560 nodes (= 8 whole graphs), split into 20 windows of 128 dst
    nodes.  Window edge lists are padded (src=0, dst_local=300) to a uniform
    number of 128-edge blocks (NBLK, global max) so one SPMD program fits
    all 8 cores; per-core behavior differs only through index inputs.
  * Per layer the device builds a node table  z_ext[n] = [z(256) | el(4) | er(4)]
    f32 (el/er are the attention logits, folded into the layer matmul via
    Wel = W @ albd, Wer = W @ arbd).  Layer 0's table is computed fully
    replicated on every core (h = x is an input); layers 1-2 compute the
    local 2560-row slice and AllGather the full table.
  * Edge phase per window: one [128,1]-indexed indirect-DMA gather per
    128-edge block pulls z_ext[src] rows (the only indirect-DMA form this
    runtime supports); er[dst] is fetched once per window for the 128 owned
    nodes and expanded to edges with a one-hot SelT matmul.  Softmax:
    ex = exp(leaky_relu(el+er)) batched per window (DVE + one ACT op);
    messages are scaled in place and scatter-added via one-hot Sel matmuls
    accumulating [out | sum_exp] in PSUM.  Per-node normalization (out/s)
    happens AFTER aggregation; the segment-max shift is dropped (softmax is
    shift-invariant and these logits cannot overflow exp in f32).
  * Pooling: per-window graph-membership one-hot matmul accumulates graph
    sums; each core emits logits for its own 8 graphs; host concatenates.
"""

import sys

import numpy as np

sys.path.insert(0, "/opt/trn_rl_repo")

import concourse.bass as bass
import concourse.bacc as bacc
import concourse.mybir as mybir
import concourse.tile as tile
from concourse.bass_utils import run_bass_kernel_spmd
from concourse.masks import make_identity

# Problem shape (hardcoded per contest rules).
N, E, G = 20480, 327680, 64
IN_DIM, H, D, C = 128, 4, 64, 10
HD = H * D            # 256
ROW = HD + 2 * H      # 264 = z | el | er
NCORES = 8
RN = N // NCORES      # 2560 dst nodes per core
P = 128
NW = RN // P          # 20 windows per core
G8 = G // NCORES      # 8 graphs per core
NEG_SLOPE = 0.2
F32 = mybir.dt.float32
F16 = mybir.dt.float16
I32 = mybir.dt.int32

TRACE = False         # set by test.py to capture HW profile
LAST_EXEC_NS = None
LAST_RESULTS = None

_CACHE = {}


def _install_ntff_hook_shim():
    """This image's ``antenv`` lacks ``axon_hooks``; provide the thin ctypes
    shim around libaxon_pjrt.so so run_bass_kernel_spmd(trace=True) works."""
    try:
        import antenv.axon_hooks  # noqa: F401
        return
    except ImportError:
        pass
    import contextlib
    import ctypes
    import types

    so_path = "/opt/axon/libaxon_pjrt.so"
    try:
        lib = ctypes.CDLL(so_path)
    except OSError:
        return
    if not hasattr(lib, "axon_start_nrt_profile"):
        return
    lib.axon_start_nrt_profile.argtypes = [ctypes.POINTER(ctypes.c_int64), ctypes.c_size_t]
    lib.axon_start_nrt_profile.restype = ctypes.c_int64
    lib.axon_stop_nrt_profile.argtypes = [ctypes.c_char_p]
    lib.axon_stop_nrt_profile.restype = ctypes.c_int64

    @contextlib.contextmanager
    def _hook(output_dir, device_ids):
        import jax

        jax.devices()
        if device_ids:
            ids = (ctypes.c_int64 * len(device_ids))(*device_ids)
            rc = lib.axon_start_nrt_profile(ids, len(device_ids))
        else:
            rc = lib.axon_start_nrt_profile(None, 0)
        if rc != 0:
            raise RuntimeError(f"axon_start_nrt_profile rc={rc}")
        try:
            yield
        finally:
            n = lib.axon_stop_nrt_profile(str(output_dir).encode())
            print(f"ntff profile: {n} file(s) written to {output_dir}")

    mod = types.ModuleType("antenv.axon_hooks")
    mod.get_axon_ntff_profile_hook = lambda: _hook
    mod.set_axon_ntff_profile_hook = lambda h: None
    sys.modules["antenv.axon_hooks"] = mod


# ----------------------------------------------------------------------------
# Host-side index preprocessing (layout only -- no arithmetic on tensor data)
# ----------------------------------------------------------------------------
def _host_prep(src, dst, graph_ids):
    order = np.argsort(dst, kind="stable")
    src_s = src[order].astype(np.int64)
    dst_s = dst[order].astype(np.int64)
    win = dst_s // P                              # global window 0..159
    cnt = np.bincount(win, minlength=NCORES * NW)
    nblk = int(np.ceil(cnt.max() / P))
    slots = nblk * P

    starts = np.zeros(NCORES * NW, np.int64)
    starts[1:] = np.cumsum(cnt)[:-1]
    srcidx = np.zeros((NCORES * NW, slots), np.int32)            # pad -> row 0
    dstloc = np.full((NCORES * NW, slots), 300.0, np.float32)    # pad -> no match
    for w in range(NCORES * NW):
        c0, c1 = starts[w], starts[w] + cnt[w]
        srcidx[w, : cnt[w]] = src_s[c0:c1]
        dstloc[w, : cnt[w]] = (dst_s[c0:c1] - w * P).astype(np.float32)

    NB = NW * nblk

    def to_cols(a, dt):
        # [160, slots] -> per-core [128, NW*nblk]; (p, w*nblk+b) = edge b*128+p
        a = a.reshape(NCORES, NW, nblk, P)
        a = np.transpose(a, (0, 3, 1, 2))
        return [
            np.ascontiguousarray(a[c].reshape(P, NB).astype(dt))
            for c in range(NCORES)
        ]

    srcidx_d = to_cols(srcidx, np.int32)
    dstloc_d = to_cols(dstloc, np.float32)
    ownid_d = [
        np.ascontiguousarray(
            (c * RN + np.arange(NW)[None, :] * P + np.arange(P)[:, None]).astype(np.int32)
        )
        for c in range(NCORES)
    ]

    gids = np.asarray(graph_ids).astype(np.int64).reshape(NCORES, NW, P)
    gmask = []
    for c in range(NCORES):
        m = np.zeros((P, NW * G8), np.float32)
        for w in range(NW):
            loc = gids[c, w] - c * G8              # 0..7 within this core
            m[np.arange(P), w * G8 + loc] = 1.0
        gmask.append(m)
    return nblk, srcidx_d, dstloc_d, ownid_d, gmask


def _blockdiag(a):
    # [H, D] -> [HD, H] block-diagonal layout so  el = z @ a_bd
    out = np.zeros((HD, H), np.float32)
    for h in range(H):
        out[h * D : (h + 1) * D, h] = a[h]
    return out


# ----------------------------------------------------------------------------
# Device program
# ----------------------------------------------------------------------------
def _build_program(nblk):
    NB = NW * nblk
    nc = bacc.Bacc(
        "TRN2",
        target_bir_lowering=False,
        debug=False,
        enable_asserts=False,
        num_devices=NCORES,
    )

    xT = nc.dram_tensor("xT", [IN_DIM, N], F32, kind="ExternalInput")
    Ws, WTs, ALs, ARs = [], [], [], []
    for l, K in enumerate([IN_DIM, HD, HD]):
        Ws.append(nc.dram_tensor(f"W{l}", [K, HD], F32, kind="ExternalInput"))
        WTs.append(nc.dram_tensor(f"WT{l}", [HD, K], F32, kind="ExternalInput"))
        ALs.append(nc.dram_tensor(f"albd{l}", [HD, H], F32, kind="ExternalInput"))
        ARs.append(nc.dram_tensor(f"arbd{l}", [HD, H], F32, kind="ExternalInput"))
    Wc = nc.dram_tensor("Wc", [HD, C], F32, kind="ExternalInput")
    bc = nc.dram_tensor("bc_rep", [G8, C], F32, kind="ExternalInput")
    srci = nc.dram_tensor("srcidx", [P, NB], I32, kind="ExternalInput")
    dstl = nc.dram_tensor("dstloc", [P, NB], F32, kind="ExternalInput")
    owni = nc.dram_tensor("ownid", [P, NW], I32, kind="ExternalInput")
    gmk = nc.dram_tensor("gmask", [P, NW * G8], F32, kind="ExternalInput")
    logits = nc.dram_tensor("logits", [G8, C], F32, kind="ExternalOutput")

    ztab = [
        nc.dram_tensor("ztab0", [N, ROW], F32),
        nc.dram_tensor("ztab1", [N, ROW], F32),
        nc.dram_tensor("ztab2", [N, ROW], F32),
    ]
    zsl = [
        None,
        nc.dram_tensor("zsl1", [RN, ROW], F32),
        nc.dram_tensor("zsl2", [RN, ROW], F32),
    ]

    AL = mybir.AluOpType

    with tile.TileContext(nc) as tc:
        with (
            tc.tile_pool(name="const", bufs=1) as constp,
            tc.tile_pool(name="wext", bufs=2) as wextp,
            tc.tile_pool(name="mm", bufs=3) as mmp,
            tc.tile_pool(name="edge", bufs=2) as edgep,
            tc.tile_pool(name="sel", bufs=2 * nblk + 2) as selp,
            tc.tile_pool(name="small", bufs=4) as smallp,
            tc.tile_pool(name="psmm", bufs=3, space="PSUM") as psmm,
            tc.tile_pool(name="psel", bufs=2, space="PSUM") as pselp,
            tc.tile_pool(name="psedge", bufs=2, space="PSUM") as psedge,
            tc.tile_pool(name="pshg", bufs=1, space="PSUM") as pshg,
        ):
            # ---- constants / resident state ----
            ident = constp.tile([P, P], F32, tag="ident")
            make_identity(nc, ident[:])
            iota_i = constp.tile([P, P], I32, tag="iota_i")
            nc.gpsimd.iota(iota_i[:], pattern=[[1, P]], base=0, channel_multiplier=0)
            iota_f = constp.tile([P, P], F32, tag="iota_f")
            nc.vector.tensor_copy(iota_f[:], iota_i[:])
            srci_sb = constp.tile([P, NB], I32, tag="srci")
            nc.sync.dma_start(srci_sb[:], srci[:, :])
            dstl_sb = constp.tile([P, NB], F32, tag="dstl")
            nc.sync.dma_start(dstl_sb[:], dstl[:, :])
            owni_sb = constp.tile([P, NW], I32, tag="owni")
            nc.sync.dma_start(owni_sb[:], owni[:, :])
            gmk_sb = constp.tile([P, NW * G8], F32, tag="gmk")
            nc.sync.dma_start(gmk_sb[:], gmk[:, :])
            h_all = constp.tile([P, NW, HD], F32, tag="h_all")
            hg_acc = constp.tile([G8, HD], F32, tag="hg_acc")
            nc.gpsimd.memset(hg_acc[:], 0.0)

            def elu_into(dst_ap, src_ap):
                # elu(x) = max(x,0) + (exp(min(x,0)) - 1)
                mn = mmp.tile([P, HD], F32, tag="emn")
                nc.vector.tensor_scalar_min(mn[:], src_ap, 0.0)
                ex = mmp.tile([P, HD], F32, tag="eex")
                nc.scalar.activation(ex[:], mn[:], mybir.ActivationFunctionType.Exp)
                mx = mmp.tile([P, HD], F32, tag="emx")
                nc.vector.tensor_scalar_max(mx[:], src_ap, 0.0)
                nc.vector.tensor_scalar_add(ex[:], ex[:], -1.0)
                nc.vector.tensor_tensor(out=dst_ap, in0=ex[:], in1=mx[:], op=AL.add)

            def build_wext(l, K):
                kch = K // P
                och = HD // P
                W_sb, WT_sb, al_sb, ar_sb = [], [], [], []
                for k in range(kch):
                    t = wextp.tile([P, HD], F32, tag="wld")
                    nc.sync.dma_start(t[:], Ws[l][k * P : (k + 1) * P, :])
                    W_sb.append(t)
                for oc in range(och):
                    t = wextp.tile([P, K], F32, tag="wtld")
                    nc.sync.dma_start(t[:], WTs[l][oc * P : (oc + 1) * P, :])
                    WT_sb.append(t)
                    ta = wextp.tile([P, H], F32, tag="alld")
                    nc.sync.dma_start(ta[:], ALs[l][oc * P : (oc + 1) * P, :])
                    al_sb.append(ta)
                    tr = wextp.tile([P, H], F32, tag="arld")
                    nc.sync.dma_start(tr[:], ARs[l][oc * P : (oc + 1) * P, :])
                    ar_sb.append(tr)
                wext = []
                for k in range(kch):
                    wx = wextp.tile([P, ROW], F32, tag="wext")
                    nc.vector.tensor_copy(wx[:, 0:HD], W_sb[k][:])
                    for dstcol, bd in ((HD, al_sb), (HD + H, ar_sb)):
                        ps = psmm.tile([P, H], F32, tag="mm")
                        for oc in range(och):
                            nc.tensor.matmul(
                                ps[:],
                                lhsT=WT_sb[oc][:, k * P : (k + 1) * P],
                                rhs=bd[oc][:],
                                start=(oc == 0),
                                stop=(oc == och - 1),
                            )
                        nc.vector.tensor_copy(wx[:, dstcol : dstcol + H], ps[:])
                    wext.append(wx)
                return wext

            def l0_table(wext):
                for t in range(N // P):
                    xt = mmp.tile([P, P], F32, tag="xt")
                    nc.sync.dma_start(xt[:], xT[:, t * P : (t + 1) * P])
                    zp = psmm.tile([P, ROW], F32, tag="mm")
                    nc.tensor.matmul(zp[:], lhsT=xt[:], rhs=wext[0][:], start=True, stop=True)
                    zs = mmp.tile([P, ROW], F32, tag="zs")
                    nc.vector.tensor_copy(zs[:], zp[:])
                    nc.sync.dma_start(ztab[0][t * P : (t + 1) * P, :], zs[:])

            def lx_slice(l, wext):
                for w in range(NW):
                    hts = []
                    for c2 in range(2):
                        tp = psmm.tile([P, P], F32, tag="mm")
                        nc.tensor.transpose(
                            tp[:], h_all[:, w, c2 * P : (c2 + 1) * P], ident[:]
                        )
                        ht = mmp.tile([P, P], F32, tag="ht")
                        nc.vector.tensor_copy(ht[:], tp[:])
                        hts.append(ht)
                    zp = psmm.tile([P, ROW], F32, tag="mm")
                    for c2 in range(2):
                        nc.tensor.matmul(
                            zp[:],
                            lhsT=hts[c2][:],
                            rhs=wext[c2][:],
                            start=(c2 == 0),
                            stop=(c2 == 1),
                        )
                    zs = mmp.tile([P, ROW], F32, tag="zs")
                    nc.vector.tensor_copy(zs[:], zp[:])
                    nc.sync.dma_start(zsl[l][w * P : (w + 1) * P, :], zs[:])
                nc.gpsimd.collective_compute(
                    "AllGather",
                    AL.bypass,
                    replica_groups=[list(range(NCORES))],
                    ins=[zsl[l][:, :]],
                    outs=[ztab[l][:, :]],
                )

            def edge_phase(l):
                for w in range(NW):
                    # er for the 128 owned dst nodes of this window
                    erw = smallp.tile([P, H], F32, tag="erw")
                    nc.gpsimd.indirect_dma_start(
                        out=erw[:],
                        out_offset=None,
                        in_=ztab[l][:, :],
                        in_offset=bass.IndirectOffsetOnAxis(
                            ap=owni_sb[:, w : w + 1], axis=0
                        ),
                        element_offset=HD + H,
                    )
                    zel = edgep.tile([P, nblk, ROW], F32, tag="zel")
                    rhs16 = edgep.tile([P, nblk, HD + H], F16, tag="rhs16")
                    eall = smallp.tile([P, nblk, H], F32, tag="eall")
                    sels = []
                    sels16 = []
                    for b in range(nblk):
                        g = w * nblk + b
                        nc.gpsimd.indirect_dma_start(
                            out=zel[:, b, :],
                            out_offset=None,
                            in_=ztab[l][:, :],
                            in_offset=bass.IndirectOffsetOnAxis(
                                ap=srci_sb[:, g : g + 1], axis=0
                            ),
                        )
                        # one-hot Sel (edges x dst-nodes); also used for the scatter
                        sel = selp.tile([P, P], F32, tag="sel")
                        nc.vector.tensor_scalar(
                            out=sel[:], in0=iota_f[:],
                            scalar1=dstl_sb[:, g : g + 1], scalar2=None,
                            op0=AL.is_equal,
                        )
                        sels.append(sel)
                        sel16 = selp.tile([P, P], F16, tag="sel16")
                        nc.vector.tensor_copy(sel16[:], sel[:])
                        sels16.append(sel16)
                        # er[dst] expansion: SelT = transpose(Sel); er_edges = SelT.T @ erw
                        stp = pselp.tile([P, P], F32, tag="psel")
                        nc.tensor.transpose(stp[:], sel[:], ident[:])
                        selt = selp.tile([P, P], F32, tag="selt")
                        nc.vector.tensor_copy(selt[:], stp[:])
                        erps = pselp.tile([P, H], F32, tag="psel")
                        nc.tensor.matmul(
                            erps[:], lhsT=selt[:], rhs=erw[:], start=True, stop=True
                        )
                        nc.vector.tensor_tensor(
                            out=eall[:, b, :], in0=zel[:, b, HD : HD + H],
                            in1=erps[:], op=AL.add,
                        )
                    # batched leaky-relu + exp for the whole window
                    et = smallp.tile([P, nblk, H], F32, tag="et")
                    nc.vector.tensor_scalar_mul(et[:], eall[:], NEG_SLOPE)
                    nc.vector.tensor_tensor(out=eall[:], in0=eall[:], in1=et[:], op=AL.max)
                    nc.scalar.activation(
                        zel[:, :, HD : HD + H], eall[:],
                        mybir.ActivationFunctionType.Exp,
                    )
                    nc.vector.tensor_copy(
                        rhs16[:, :, HD : HD + H], zel[:, :, HD : HD + H]
                    )
                    outp = psedge.tile([P, HD + H], F32, tag="outp")
                    for b in range(nblk):
                        sel = sels[b]
                        nc.vector.tensor_tensor(
                            out=rhs16[:, b, 0:HD].rearrange("p (h d) -> p h d", h=H),
                            in0=zel[:, b, 0:HD].rearrange("p (h d) -> p h d", h=H),
                            in1=zel[:, b, HD : HD + H].to_broadcast([P, H, D]),
                            op=AL.mult,
                        )
                        nc.tensor.matmul(
                            outp[:],
                            lhsT=sels16[b][:],
                            rhs=rhs16[:, b, :],
                            start=(b == 0),
                            stop=(b == nblk - 1),
                        )
                    # normalize + activations
                    rec = smallp.tile([P, H], F32, tag="rec")
                    nc.vector.reciprocal(rec[:], outp[:, HD : HD + H])
                    agg = mmp.tile([P, HD], F32, tag="agg")
                    nc.vector.tensor_tensor(
                        out=agg[:].rearrange("p (h d) -> p h d", h=H),
                        in0=outp[:, 0:HD].rearrange("p (h d) -> p h d", h=H),
                        in1=rec[:].to_broadcast([P, H, D]),
                        op=AL.mult,
                    )
                    if l == 0:
                        elu_into(h_all[:, w, :], agg[:])
                    else:
                        nc.vector.tensor_tensor(
                            out=agg[:], in0=agg[:], in1=h_all[:, w, :], op=AL.add
                        )
                        tmp = mmp.tile([P, HD], F32, tag="agg2")
                        elu_into(tmp[:], agg[:])
                        elu_into(h_all[:, w, :], tmp[:])
                    if l == 2:
                        gp = pshg.tile([G8, HD], F32, tag="hg")
                        nc.tensor.matmul(
                            gp[:],
                            lhsT=gmk_sb[:, w * G8 : (w + 1) * G8],
                            rhs=h_all[:, w, :],
                            start=True,
                            stop=True,
                        )
                        nc.vector.tensor_tensor(
                            out=hg_acc[:], in0=hg_acc[:], in1=gp[:], op=AL.add
                        )

            # ---- layer 0 ----
            wext0 = build_wext(0, IN_DIM)
            l0_table(wext0)
            edge_phase(0)
            # ---- layers 1, 2 ----
            for l in (1, 2):
                wextl = build_wext(l, HD)
                lx_slice(l, wextl)
                edge_phase(l)

            # ---- pooling epilogue: hg -> elu -> @Wc + bc ----
            hg_sb = smallp.tile([G8, HD], F32, tag="hg_sb")
            nc.vector.tensor_scalar_mul(hg_sb[:], hg_acc[:], 1.0 / (N // G))
            mn = smallp.tile([G8, HD], F32, tag="fmn")
            nc.vector.tensor_scalar_min(mn[:], hg_sb[:], 0.0)
            exx = smallp.tile([G8, HD], F32, tag="fex")
            nc.scalar.activation(exx[:], mn[:], mybir.ActivationFunctionType.Exp)
            mx = smallp.tile([G8, HD], F32, tag="fmx")
            nc.vector.tensor_scalar_max(mx[:], hg_sb[:], 0.0)
            nc.vector.tensor_scalar_add(exx[:], exx[:], -1.0)
            nc.vector.tensor_tensor(out=hg_sb[:], in0=exx[:], in1=mx[:], op=AL.add)

            wc_sb, hgts = [], []
            for c2 in range(2):
                t = smallp.tile([P, C], F32, tag="wc")
                nc.sync.dma_start(t[:], Wc[c2 * P : (c2 + 1) * P, :])
                wc_sb.append(t)
                tp = psmm.tile([P, G8], F32, tag="mm")
                nc.tensor.transpose(
                    tp[:], hg_sb[:, c2 * P : (c2 + 1) * P], ident[:G8, :G8]
                )
                hgt = smallp.tile([P, G8], F32, tag="hgt")
                nc.vector.tensor_copy(hgt[:], tp[:])
                hgts.append(hgt)
            lg = psmm.tile([G8, C], F32, tag="mm")
            for c2 in range(2):
                nc.tensor.matmul(
                    lg[:], lhsT=hgts[c2][:], rhs=wc_sb[c2][:],
                    start=(c2 == 0), stop=(c2 == 1),
                )
            bc_sb = smallp.tile([G8, C], F32, tag="bc")
            nc.sync.dma_start(bc_sb[:], bc[:, :])
            lg_sb = smallp.tile([G8, C], F32, tag="lg")
            nc.vector.tensor_tensor(out=lg_sb[:], in0=lg[:], in1=bc_sb[:], op=AL.add)
            nc.sync.dma_start(logits[:, :], lg_sb[:])

    nc.compile()
    return nc


def _get_program(nblk):
    if nblk not in _CACHE:
        _CACHE[nblk] = _build_program(nblk)
    return _CACHE[nblk]


# ----------------------------------------------------------------------------
# Entry point
# ----------------------------------------------------------------------------
def kernel(x, src, dst, graph_ids, W0, al0, ar0, W1, al1, ar1, W2, al2, ar2, Wc, bc):
    global LAST_EXEC_NS, LAST_RESULTS
    x = np.ascontiguousarray(np.asarray(x, np.float32))
    src = np.asarray(src).astype(np.int32)
    dst = np.asarray(dst).astype(np.int32)
    graph_ids = np.asarray(graph_ids).astype(np.int32)

    nblk, srcidx_d, dstloc_d, ownid_d, gmask_d = _host_prep(src, dst, graph_ids)
    nc = _get_program(nblk)

    xT = np.ascontiguousarray(x.T)
    Wl = [np.asarray(W0, np.float32), np.asarray(W1, np.float32), np.asarray(W2, np.float32)]
    als = [al0, al1, al2]
    ars = [ar0, ar1, ar2]
    common = {"xT": xT, "Wc": np.asarray(Wc, np.float32),
              "bc_rep": np.tile(np.asarray(bc, np.float32)[None, :], (G8, 1))}
    for l in range(3):
        common[f"W{l}"] = Wl[l]
        common[f"WT{l}"] = np.ascontiguousarray(Wl[l].T)
        common[f"albd{l}"] = _blockdiag(np.asarray(als[l], np.float32))
        common[f"arbd{l}"] = _blockdiag(np.asarray(ars[l], np.float32))

    in_maps = []
    for c in range(NCORES):
        m = dict(common)
        m["srcidx"] = srcidx_d[c]
        m["dstloc"] = dstloc_d[c]
        m["ownid"] = ownid_d[c]
        m["gmask"] = gmask_d[c]
        in_maps.append(m)

    if TRACE:
        _install_ntff_hook_shim()
    res = run_bass_kernel_spmd(nc, in_maps, list(range(NCORES)), trace=TRACE)
    LAST_EXEC_NS = res.exec_time_ns
    LAST_RESULTS = res
    out = np.concatenate([res.results[c]["logits"] for c in range(NCORES)], axis=0)
    return out.astype(np.float32)



# revision 9
# speedup vs baseline: 1.0367x; 1.0084x over previous
"""3-layer GAT + per-graph mean-pool + linear head, distributed over 8 NeuronCores.

Strategy (edge-parallel, dst-sorted):
  * Host: sort edges by dst; each core owns a contiguous dst range of
    N/8 = 2560 nodes (= 8 whole graphs), split into 20 windows of 128 dst
    nodes.  Window edge lists are padded (src=0, dst_local=300) to a uniform
    number of 128-edge blocks (NBLK, global max) so one SPMD program fits
    all 8 cores; per-core behavior differs only through index inputs.
  * Per layer the device builds a node table  z_ext[n] = [z(256) | el(4) | er(4)]
    f32 (el/er are the attention logits, folded into the layer matmul via
    Wel = W @ albd, Wer = W @ arbd).  Layer 0's table is computed fully
    replicated on every core (h = x is an input); layers 1-2 compute the
    local 2560-row slice and AllGather the full table.
  * Edge phase per window: one [128,1]-indexed indirect-DMA gather per
    128-edge block pulls z_ext[src] rows (the only indirect-DMA form this
    runtime supports); er[dst] is fetched once per window for the 128 owned
    nodes and expanded to edges with a one-hot SelT matmul.  Softmax:
    ex = exp(leaky_relu(el+er)) batched per window (DVE + one ACT op);
    messages are scaled in place and scatter-added via one-hot Sel matmuls
    accumulating [out | sum_exp] in PSUM.  Per-node normalization (out/s)
    happens AFTER aggregation; the segment-max shift is dropped (softmax is
    shift-invariant and these logits cannot overflow exp in f32).
  * Pooling: per-window graph-membership one-hot matmul accumulates graph
    sums; each core emits logits for its own 8 graphs; host concatenates.
"""

import sys

import numpy as np

sys.path.insert(0, "/opt/trn_rl_repo")

import concourse.bass as bass
import concourse.bacc as bacc
import concourse.mybir as mybir
import concourse.tile as tile
from concourse.bass_utils import run_bass_kernel_spmd
from concourse.masks import make_identity

# Problem shape (hardcoded per contest rules).
N, E, G = 20480, 327680, 64
IN_DIM, H, D, C = 128, 4, 64, 10
HD = H * D            # 256
ROW = HD + 2 * H      # 264 = z | el | er
NCORES = 8
RN = N // NCORES      # 2560 dst nodes per core
P = 128
NW = RN // P          # 20 windows per core
G8 = G // NCORES      # 8 graphs per core
NEG_SLOPE = 0.2
F32 = mybir.dt.float32
F16 = mybir.dt.float16
I32 = mybir.dt.int32

TRACE = False         # set by test.py to capture HW profile
LAST_EXEC_NS = None
LAST_RESULTS = None

_CACHE = {}


def _install_ntff_hook_shim():
    """This image's ``antenv`` lacks ``axon_hooks``; provide the thin ctypes
    shim around libaxon_pjrt.so so run_bass_kernel_spmd(trace=True) works."""
    try:
        import antenv.axon_hooks  # noqa: F401
        return
    except ImportError:
        pass
    import contextlib
    import ctypes
    import types

    so_path = "/opt/axon/libaxon_pjrt.so"
    try:
        lib = ctypes.CDLL(so_path)
    except OSError:
        return
    if not hasattr(lib, "axon_start_nrt_profile"):
        return
    lib.axon_start_nrt_profile.argtypes = [ctypes.POINTER(ctypes.c_int64), ctypes.c_size_t]
    lib.axon_start_nrt_profile.restype = ctypes.c_int64
    lib.axon_stop_nrt_profile.argtypes = [ctypes.c_char_p]
    lib.axon_stop_nrt_profile.restype = ctypes.c_int64

    @contextlib.contextmanager
    def _hook(output_dir, device_ids):
        import jax

        jax.devices()
        if device_ids:
            ids = (ctypes.c_int64 * len(device_ids))(*device_ids)
            rc = lib.axon_start_nrt_profile(ids, len(device_ids))
        else:
            rc = lib.axon_start_nrt_profile(None, 0)
        if rc != 0:
            raise RuntimeError(f"axon_start_nrt_profile rc={rc}")
        try:
            yield
        finally:
            n = lib.axon_stop_nrt_profile(str(output_dir).encode())
            print(f"ntff profile: {n} file(s) written to {output_dir}")

    mod = types.ModuleType("antenv.axon_hooks")
    mod.get_axon_ntff_profile_hook = lambda: _hook
    mod.set_axon_ntff_profile_hook = lambda h: None
    sys.modules["antenv.axon_hooks"] = mod


# ----------------------------------------------------------------------------
# Host-side index preprocessing (layout only -- no arithmetic on tensor data)
# ----------------------------------------------------------------------------
def _host_prep(src, dst, graph_ids):
    order = np.argsort(dst, kind="stable")
    src_s = src[order].astype(np.int64)
    dst_s = dst[order].astype(np.int64)
    win = dst_s // P                              # global window 0..159
    cnt = np.bincount(win, minlength=NCORES * NW)
    nblk = int(np.ceil(cnt.max() / P))
    slots = nblk * P

    starts = np.zeros(NCORES * NW, np.int64)
    starts[1:] = np.cumsum(cnt)[:-1]
    srcidx = np.zeros((NCORES * NW, slots), np.int32)            # pad -> row 0
    dstloc = np.full((NCORES * NW, slots), 300.0, np.float32)    # pad -> no match
    for w in range(NCORES * NW):
        c0, c1 = starts[w], starts[w] + cnt[w]
        srcidx[w, : cnt[w]] = src_s[c0:c1]
        dstloc[w, : cnt[w]] = (dst_s[c0:c1] - w * P).astype(np.float32)

    NB = NW * nblk

    def to_cols(a, dt):
        # [160, slots] -> per-core [128, NW*nblk]; (p, w*nblk+b) = edge b*128+p
        a = a.reshape(NCORES, NW, nblk, P)
        a = np.transpose(a, (0, 3, 1, 2))
        return [
            np.ascontiguousarray(a[c].reshape(P, NB).astype(dt))
            for c in range(NCORES)
        ]

    srcidx_d = to_cols(srcidx, np.int32)
    dstloc_d = to_cols(dstloc, np.float32)
    ownid_d = [
        np.ascontiguousarray(
            (c * RN + np.arange(NW)[None, :] * P + np.arange(P)[:, None]).astype(np.int32)
        )
        for c in range(NCORES)
    ]

    gids = np.asarray(graph_ids).astype(np.int64).reshape(NCORES, NW, P)
    gmask = []
    for c in range(NCORES):
        m = np.zeros((P, NW * G8), np.float32)
        for w in range(NW):
            loc = gids[c, w] - c * G8              # 0..7 within this core
            m[np.arange(P), w * G8 + loc] = 1.0
        gmask.append(m)
    return nblk, srcidx_d, dstloc_d, ownid_d, gmask


def _blockdiag(a):
    # [H, D] -> [HD, H] block-diagonal layout so  el = z @ a_bd
    out = np.zeros((HD, H), np.float32)
    for h in range(H):
        out[h * D : (h + 1) * D, h] = a[h]
    return out


# ----------------------------------------------------------------------------
# Device program
# ----------------------------------------------------------------------------
def _build_program(nblk):
    NB = NW * nblk
    nc = bacc.Bacc(
        "TRN2",
        target_bir_lowering=False,
        debug=False,
        enable_asserts=False,
        num_devices=NCORES,
    )

    xT = nc.dram_tensor("xT", [IN_DIM, N], F32, kind="ExternalInput")
    Ws, WTs, ALs, ARs = [], [], [], []
    for l, K in enumerate([IN_DIM, HD, HD]):
        Ws.append(nc.dram_tensor(f"W{l}", [K, HD], F32, kind="ExternalInput"))
        WTs.append(nc.dram_tensor(f"WT{l}", [HD, K], F32, kind="ExternalInput"))
        ALs.append(nc.dram_tensor(f"albd{l}", [HD, H], F32, kind="ExternalInput"))
        ARs.append(nc.dram_tensor(f"arbd{l}", [HD, H], F32, kind="ExternalInput"))
    Wc = nc.dram_tensor("Wc", [HD, C], F32, kind="ExternalInput")
    bc = nc.dram_tensor("bc_rep", [G8, C], F32, kind="ExternalInput")
    srci = nc.dram_tensor("srcidx", [P, NB], I32, kind="ExternalInput")
    dstl = nc.dram_tensor("dstloc", [P, NB], F32, kind="ExternalInput")
    owni = nc.dram_tensor("ownid", [P, NW], I32, kind="ExternalInput")
    gmk = nc.dram_tensor("gmask", [P, NW * G8], F32, kind="ExternalInput")
    logits = nc.dram_tensor("logits", [G8, C], F32, kind="ExternalOutput")

    ztab = [
        nc.dram_tensor("ztab0", [N, ROW], F32),
        nc.dram_tensor("ztab1", [N, ROW], F32),
        nc.dram_tensor("ztab2", [N, ROW], F32),
    ]
    zsl = [
        None,
        nc.dram_tensor("zsl1", [RN, ROW], F32),
        nc.dram_tensor("zsl2", [RN, ROW], F32),
    ]

    AL = mybir.AluOpType

    with tile.TileContext(nc) as tc:
        with (
            tc.tile_pool(name="const", bufs=1) as constp,
            tc.tile_pool(name="wext", bufs=2) as wextp,
            tc.tile_pool(name="mm", bufs=3) as mmp,
            tc.tile_pool(name="edge", bufs=3) as edgep,
            tc.tile_pool(name="sel", bufs=2 * nblk + 2) as selp,
            tc.tile_pool(name="small", bufs=4) as smallp,
            tc.tile_pool(name="psmm", bufs=3, space="PSUM") as psmm,
            tc.tile_pool(name="psel", bufs=2, space="PSUM") as pselp,
            tc.tile_pool(name="psedge", bufs=2, space="PSUM") as psedge,
            tc.tile_pool(name="pshg", bufs=1, space="PSUM") as pshg,
        ):
            # ---- constants / resident state ----
            ident = constp.tile([P, P], F32, tag="ident")
            make_identity(nc, ident[:])
            iota_i = constp.tile([P, P], I32, tag="iota_i")
            nc.gpsimd.iota(iota_i[:], pattern=[[1, P]], base=0, channel_multiplier=0)
            iota_f = constp.tile([P, P], F32, tag="iota_f")
            nc.vector.tensor_copy(iota_f[:], iota_i[:])
            srci_sb = constp.tile([P, NB], I32, tag="srci")
            nc.sync.dma_start(srci_sb[:], srci[:, :])
            dstl_sb = constp.tile([P, NB], F32, tag="dstl")
            nc.sync.dma_start(dstl_sb[:], dstl[:, :])
            owni_sb = constp.tile([P, NW], I32, tag="owni")
            nc.sync.dma_start(owni_sb[:], owni[:, :])
            gmk_sb = constp.tile([P, NW * G8], F32, tag="gmk")
            nc.sync.dma_start(gmk_sb[:], gmk[:, :])
            h_all = constp.tile([P, NW, HD], F32, tag="h_all")
            hg_acc = constp.tile([G8, HD], F32, tag="hg_acc")
            nc.gpsimd.memset(hg_acc[:], 0.0)

            def elu_into(dst_ap, src_ap):
                # elu(x) = max(x,0) + (exp(min(x,0)) - 1)
                mn = mmp.tile([P, HD], F32, tag="emn")
                nc.vector.tensor_scalar_min(mn[:], src_ap, 0.0)
                ex = mmp.tile([P, HD], F32, tag="eex")
                nc.scalar.activation(ex[:], mn[:], mybir.ActivationFunctionType.Exp)
                mx = mmp.tile([P, HD], F32, tag="emx")
                nc.vector.tensor_scalar_max(mx[:], src_ap, 0.0)
                nc.vector.tensor_scalar_add(ex[:], ex[:], -1.0)
                nc.vector.tensor_tensor(out=dst_ap, in0=ex[:], in1=mx[:], op=AL.add)

            def build_wext(l, K):
                kch = K // P
                och = HD // P
                W_sb, WT_sb, al_sb, ar_sb = [], [], [], []
                for k in range(kch):
                    t = wextp.tile([P, HD], F32, tag="wld")
                    nc.sync.dma_start(t[:], Ws[l][k * P : (k + 1) * P, :])
                    W_sb.append(t)
                for oc in range(och):
                    t = wextp.tile([P, K], F32, tag="wtld")
                    nc.sync.dma_start(t[:], WTs[l][oc * P : (oc + 1) * P, :])
                    WT_sb.append(t)
                    ta = wextp.tile([P, H], F32, tag="alld")
                    nc.sync.dma_start(ta[:], ALs[l][oc * P : (oc + 1) * P, :])
                    al_sb.append(ta)
                    tr = wextp.tile([P, H], F32, tag="arld")
                    nc.sync.dma_start(tr[:], ARs[l][oc * P : (oc + 1) * P, :])
                    ar_sb.append(tr)
                wext = []
                for k in range(kch):
                    wx = wextp.tile([P, ROW], F32, tag="wext")
                    nc.vector.tensor_copy(wx[:, 0:HD], W_sb[k][:])
                    for dstcol, bd in ((HD, al_sb), (HD + H, ar_sb)):
                        ps = psmm.tile([P, H], F32, tag="mm")
                        for oc in range(och):
                            nc.tensor.matmul(
                                ps[:],
                                lhsT=WT_sb[oc][:, k * P : (k + 1) * P],
                                rhs=bd[oc][:],
                                start=(oc == 0),
                                stop=(oc == och - 1),
                            )
                        nc.vector.tensor_copy(wx[:, dstcol : dstcol + H], ps[:])
                    wext.append(wx)
                return wext

            def l0_table(wext):
                for t in range(N // P):
                    xt = mmp.tile([P, P], F32, tag="xt")
                    nc.sync.dma_start(xt[:], xT[:, t * P : (t + 1) * P])
                    zp = psmm.tile([P, ROW], F32, tag="mm")
                    nc.tensor.matmul(zp[:], lhsT=xt[:], rhs=wext[0][:], start=True, stop=True)
                    zs = mmp.tile([P, ROW], F32, tag="zs")
                    nc.vector.tensor_copy(zs[:], zp[:])
                    nc.sync.dma_start(ztab[0][t * P : (t + 1) * P, :], zs[:])

            def lx_slice(l, wext):
                for w in range(NW):
                    hts = []
                    for c2 in range(2):
                        tp = psmm.tile([P, P], F32, tag="mm")
                        nc.tensor.transpose(
                            tp[:], h_all[:, w, c2 * P : (c2 + 1) * P], ident[:]
                        )
                        ht = mmp.tile([P, P], F32, tag="ht")
                        nc.vector.tensor_copy(ht[:], tp[:])
                        hts.append(ht)
                    zp = psmm.tile([P, ROW], F32, tag="mm")
                    for c2 in range(2):
                        nc.tensor.matmul(
                            zp[:],
                            lhsT=hts[c2][:],
                            rhs=wext[c2][:],
                            start=(c2 == 0),
                            stop=(c2 == 1),
                        )
                    zs = mmp.tile([P, ROW], F32, tag="zs")
                    nc.vector.tensor_copy(zs[:], zp[:])
                    nc.sync.dma_start(zsl[l][w * P : (w + 1) * P, :], zs[:])
                nc.gpsimd.collective_compute(
                    "AllGather",
                    AL.bypass,
                    replica_groups=[list(range(NCORES))],
                    ins=[zsl[l][:, :]],
                    outs=[ztab[l][:, :]],
                )

            def edge_phase(l):
                for w in range(NW):
                    # er for the 128 owned dst nodes of this window
                    erw = smallp.tile([P, H], F32, tag="erw")
                    nc.gpsimd.indirect_dma_start(
                        out=erw[:],
                        out_offset=None,
                        in_=ztab[l][:, :],
                        in_offset=bass.IndirectOffsetOnAxis(
                            ap=owni_sb[:, w : w + 1], axis=0
                        ),
                        element_offset=HD + H,
                    )
                    zel = edgep.tile([P, nblk, ROW], F32, tag="zel")
                    rhs16 = edgep.tile([P, nblk, HD + H], F16, tag="rhs16")
                    eall = smallp.tile([P, nblk, H], F32, tag="eall")
                    sels = []
                    sels16 = []
                    for b in range(nblk):
                        g = w * nblk + b
                        nc.gpsimd.indirect_dma_start(
                            out=zel[:, b, :],
                            out_offset=None,
                            in_=ztab[l][:, :],
                            in_offset=bass.IndirectOffsetOnAxis(
                                ap=srci_sb[:, g : g + 1], axis=0
                            ),
                        )
                        # one-hot Sel (edges x dst-nodes); also used for the scatter
                        sel = selp.tile([P, P], F32, tag="sel")
                        nc.vector.tensor_scalar(
                            out=sel[:], in0=iota_f[:],
                            scalar1=dstl_sb[:, g : g + 1], scalar2=None,
                            op0=AL.is_equal,
                        )
                        sels.append(sel)
                        sel16 = selp.tile([P, P], F16, tag="sel16")
                        nc.vector.tensor_copy(sel16[:], sel[:])
                        sels16.append(sel16)
                        # er[dst] expansion: SelT = transpose(Sel); er_edges = SelT.T @ erw
                        stp = pselp.tile([P, P], F32, tag="psel")
                        nc.tensor.transpose(stp[:], sel[:], ident[:])
                        selt = selp.tile([P, P], F32, tag="selt")
                        nc.vector.tensor_copy(selt[:], stp[:])
                        erps = pselp.tile([P, H], F32, tag="psel")
                        nc.tensor.matmul(
                            erps[:], lhsT=selt[:], rhs=erw[:], start=True, stop=True
                        )
                        nc.vector.tensor_tensor(
                            out=eall[:, b, :], in0=zel[:, b, HD : HD + H],
                            in1=erps[:], op=AL.add,
                        )
                    # batched leaky-relu + exp for the whole window
                    et = smallp.tile([P, nblk, H], F32, tag="et")
                    nc.vector.tensor_scalar_mul(et[:], eall[:], NEG_SLOPE)
                    nc.vector.tensor_tensor(out=eall[:], in0=eall[:], in1=et[:], op=AL.max)
                    nc.scalar.activation(
                        zel[:, :, HD : HD + H], eall[:],
                        mybir.ActivationFunctionType.Exp,
                    )
                    nc.vector.tensor_copy(
                        rhs16[:, :, HD : HD + H], zel[:, :, HD : HD + H]
                    )
                    outp = psedge.tile([P, HD + H], F32, tag="outp")
                    for b in range(nblk):
                        sel = sels[b]
                        nc.vector.tensor_tensor(
                            out=rhs16[:, b, 0:HD].rearrange("p (h d) -> p h d", h=H),
                            in0=zel[:, b, 0:HD].rearrange("p (h d) -> p h d", h=H),
                            in1=zel[:, b, HD : HD + H].to_broadcast([P, H, D]),
                            op=AL.mult,
                        )
                        nc.tensor.matmul(
                            outp[:],
                            lhsT=sels16[b][:],
                            rhs=rhs16[:, b, :],
                            start=(b == 0),
                            stop=(b == nblk - 1),
                        )
                    # normalize + activations
                    rec = smallp.tile([P, H], F32, tag="rec")
                    nc.vector.reciprocal(rec[:], outp[:, HD : HD + H])
                    agg = mmp.tile([P, HD], F32, tag="agg")
                    nc.vector.tensor_tensor(
                        out=agg[:].rearrange("p (h d) -> p h d", h=H),
                        in0=outp[:, 0:HD].rearrange("p (h d) -> p h d", h=H),
                        in1=rec[:].to_broadcast([P, H, D]),
                        op=AL.mult,
                    )
                    if l == 0:
                        elu_into(h_all[:, w, :], agg[:])
                    else:
                        nc.vector.tensor_tensor(
                            out=agg[:], in0=agg[:], in1=h_all[:, w, :], op=AL.add
                        )
                        tmp = mmp.tile([P, HD], F32, tag="agg2")
                        elu_into(tmp[:], agg[:])
                        elu_into(h_all[:, w, :], tmp[:])
                    if l == 2:
                        gp = pshg.tile([G8, HD], F32, tag="hg")
                        nc.tensor.matmul(
                            gp[:],
                            lhsT=gmk_sb[:, w * G8 : (w + 1) * G8],
                            rhs=h_all[:, w, :],
                            start=True,
                            stop=True,
                        )
                        nc.vector.tensor_tensor(
                            out=hg_acc[:], in0=hg_acc[:], in1=gp[:], op=AL.add
                        )

            # ---- layer 0 ----
            wext0 = build_wext(0, IN_DIM)
            l0_table(wext0)
            edge_phase(0)
            # ---- layers 1, 2 ----
            for l in (1, 2):
                wextl = build_wext(l, HD)
                lx_slice(l, wextl)
                edge_phase(l)

            # ---- pooling epilogue: hg -> elu -> @Wc + bc ----
            hg_sb = smallp.tile([G8, HD], F32, tag="hg_sb")
            nc.vector.tensor_scalar_mul(hg_sb[:], hg_acc[:], 1.0 / (N // G))
            mn = smallp.tile([G8, HD], F32, tag="fmn")
            nc.vector.tensor_scalar_min(mn[:], hg_sb[:], 0.0)
            exx = smallp.tile([G8, HD], F32, tag="fex")
            nc.scalar.activation(exx[:], mn[:], mybir.ActivationFunctionType.Exp)
            mx = smallp.tile([G8, HD], F32, tag="fmx")
            nc.vector.tensor_scalar_max(mx[:], hg_sb[:], 0.0)
            nc.vector.tensor_scalar_add(exx[:], exx[:], -1.0)
            nc.vector.tensor_tensor(out=hg_sb[:], in0=exx[:], in1=mx[:], op=AL.add)

            wc_sb, hgts = [], []
            for c2 in range(2):
                t = smallp.tile([P, C], F32, tag="wc")
                nc.sync.dma_start(t[:], Wc[c2 * P : (c2 + 1) * P, :])
                wc_sb.append(t)
                tp = psmm.tile([P, G8], F32, tag="mm")
                nc.tensor.transpose(
                    tp[:], hg_sb[:, c2 * P : (c2 + 1) * P], ident[:G8, :G8]
                )
                hgt = smallp.tile([P, G8], F32, tag="hgt")
                nc.vector.tensor_copy(hgt[:], tp[:])
                hgts.append(hgt)
            lg = psmm.tile([G8, C], F32, tag="mm")
            for c2 in range(2):
                nc.tensor.matmul(
                    lg[:], lhsT=hgts[c2][:], rhs=wc_sb[c2][:],
                    start=(c2 == 0), stop=(c2 == 1),
                )
            bc_sb = smallp.tile([G8, C], F32, tag="bc")
            nc.sync.dma_start(bc_sb[:], bc[:, :])
            lg_sb = smallp.tile([G8, C], F32, tag="lg")
            nc.vector.tensor_tensor(out=lg_sb[:], in0=lg[:], in1=bc_sb[:], op=AL.add)
            nc.sync.dma_start(logits[:, :], lg_sb[:])

    nc.compile()
    return nc


def _get_program(nblk):
    if nblk not in _CACHE:
        _CACHE[nblk] = _build_program(nblk)
    return _CACHE[nblk]


# ----------------------------------------------------------------------------
# Entry point
# ----------------------------------------------------------------------------
def kernel(x, src, dst, graph_ids, W0, al0, ar0, W1, al1, ar1, W2, al2, ar2, Wc, bc):
    global LAST_EXEC_NS, LAST_RESULTS
    x = np.ascontiguousarray(np.asarray(x, np.float32))
    src = np.asarray(src).astype(np.int32)
    dst = np.asarray(dst).astype(np.int32)
    graph_ids = np.asarray(graph_ids).astype(np.int32)

    nblk, srcidx_d, dstloc_d, ownid_d, gmask_d = _host_prep(src, dst, graph_ids)
    nc = _get_program(nblk)

    xT = np.ascontiguousarray(x.T)
    Wl = [np.asarray(W0, np.float32), np.asarray(W1, np.float32), np.asarray(W2, np.float32)]
    als = [al0, al1, al2]
    ars = [ar0, ar1, ar2]
    common = {"xT": xT, "Wc": np.asarray(Wc, np.float32),
              "bc_rep": np.tile(np.asarray(bc, np.float32)[None, :], (G8, 1))}
    for l in range(3):
        common[f"W{l}"] = Wl[l]
        common[f"WT{l}"] = np.ascontiguousarray(Wl[l].T)
        common[f"albd{l}"] = _blockdiag(np.asarray(als[l], np.float32))
        common[f"arbd{l}"] = _blockdiag(np.asarray(ars[l], np.float32))

    in_maps = []
    for c in range(NCORES):
        m = dict(common)
        m["srcidx"] = srcidx_d[c]
        m["dstloc"] = dstloc_d[c]
        m["ownid"] = ownid_d[c]
        m["gmask"] = gmask_d[c]
        in_maps.append(m)

    if TRACE:
        _install_ntff_hook_shim()
    res = run_bass_kernel_spmd(nc, in_maps, list(range(NCORES)), trace=TRACE)
    LAST_EXEC_NS = res.exec_time_ns
    LAST_RESULTS = res
    out = np.concatenate([res.results[c]["logits"] for c in range(NCORES)], axis=0)
    return out.astype(np.float32)

